# revision 25
# baseline (speedup 1.0000x reference)
"""MoE ExpertLayer kernel for Trainium2 (8 NeuronCores, data-parallel over tokens).

Reference computation (B=4, S=2048, D=1024, E=8):
    logits  = x @ W_router.T + b_router          # [B,S,E]
    probs   = softmax(logits, axis=-1)
    y_e     = x @ W_experts[e].T + b_experts[e]  # all experts, dense
    out     = sum_e probs[..., e] * y_e          # [B,S,D]

Sharding: data-parallel over the flattened token axis (8192 tokens -> 1024
tokens per core). Every core receives the full (transposed) expert weights and
computes its token shard end-to-end; no collectives are needed.

Per-core dataflow:
  - xT [D, T] resident in SBUF; expert weights streamed one expert at a time
    as WtT[e] = W_experts[e].T (so the contraction dim d lands on SBUF
    partitions for both matmul operands with contiguous DMA).
  - Router: 8 accumulating matmuls per token tile -> PSUM [128 tok, 8 e],
    + K=1 ones-matmul to add b_router; softmax via DVE reduce_max(negate) +
    ACT Exp(bias=-max, accum_out=sum) + DVE reciprocal + tensor_scalar_mul.
  - Bias fold: out bias term sum_e probs[t,e]*b_e[f] is a K=8 matmul
    (probs.T as stationary) accumulated straight into the output accumulator.
  - Experts: psum[t=128, f=512] accumulates 8 d-tile matmuls; the combine
    acc = psum * probs[:,e] + acc is one fused DVE scalar_tensor_tensor op.
"""

import os
import sys

for _p in ("/opt/trn_rl_repo", "/root/.axon_site/_ro/trn_rl_repo"):
    if os.path.isdir(_p) and _p not in sys.path:
        sys.path.insert(0, _p)

from contextlib import ExitStack

import ml_dtypes
import numpy as np

import concourse.bass as bass
import concourse.mybir as mybir
import concourse.tile as tile
from concourse import bacc
from concourse.bass import ts
from concourse.bass_utils import run_bass_kernel_spmd
from concourse.masks import make_identity

B, S, D, E = 4, 2048, 1024, 8
N_CORES = 8
T = B * S // N_CORES  # tokens per core = 1024
P = 128               # partitions
TT = T // P           # token tiles per core = 8
DT = D // P           # contraction tiles = 8
FN = 512              # matmul moving free dim (one PSUM bank of fp32)
FH = D // FN          # output column halves = 2

MODE = os.environ.get("KERNEL_MODE", "bf16")  # bf16 | f32r | f32


def _compute_dt(mode):
    return {
        "bf16": mybir.dt.bfloat16,
        "f32r": mybir.dt.float32r,
        "f32": mybir.dt.float32,
    }[mode]


def _np_dt(mode):
    return {"bf16": ml_dtypes.bfloat16, "f32r": np.float32, "f32": np.float32}[mode]


def build(mode=MODE):
    """Build the per-core Bass/Tile program (identical SPMD program on all cores)."""
    cdt = _compute_dt(mode)
    f32 = mybir.dt.float32

    nc = bacc.Bacc("TRN2", target_bir_lowering=False, debug=False)

    # Inputs are pre-tiled on the host to [partition, ..., d-tile, ...] so
    # every DMA reads long contiguous per-partition chunks — the naive
    # [D, ...] layout yields 2KB strided descriptors that throttle a HWDGE
    # queue. xT is additionally split by token half so the router can start
    # as soon as the first half lands.
    TH = 2          # token halves per core
    THT = T // TH   # 512 tokens per half
    xT_d = nc.dram_tensor("xT", [P, TH, DT, THT], cdt, kind="ExternalInput").ap()
    Wt_d = nc.dram_tensor("Wt", [E, P, DT, D], cdt, kind="ExternalInput").ap()
    be_d = nc.dram_tensor("be", [E, D], cdt, kind="ExternalInput").ap()
    WrT_d = nc.dram_tensor("WrT", [P, DT, E], cdt, kind="ExternalInput").ap()
    brT_d = nc.dram_tensor("brT", [E, 1], f32, kind="ExternalInput").ap()
    out_d = nc.dram_tensor("out", [T, D], f32, kind="ExternalOutput").ap()

    with tile.TileContext(nc) as tc, ExitStack() as ctx:
        singles = ctx.enter_context(tc.tile_pool(name="singles", bufs=1))
        wpool = ctx.enter_context(tc.tile_pool(name="wpool", bufs=3))
        small = ctx.enter_context(tc.tile_pool(name="small", bufs=4))
        ppool = ctx.enter_context(tc.tile_pool(name="psum_e", bufs=2, space="PSUM"))
        pbias = ctx.enter_context(tc.tile_pool(name="psum_b", bufs=1, space="PSUM"))
        prout = ctx.enter_context(tc.tile_pool(name="psum_r", bufs=1, space="PSUM"))

        # Two HWDGE rings (sync=SP, scalar=ACT): spread big transfers across
        # both — a single ring saturates around ~120 GB/s for these patterns.
        hwdge = [nc.sync, nc.scalar]

        # Resident tensors. Tiny router tensors go first (they gate the router
        # phase), then the xT token-halves — one per ring — then weights.
        WrT = singles.tile([P, DT, E], cdt)
        nc.sync.dma_start(out=WrT, in_=WrT_d)
        brT = singles.tile([E, 1], f32)
        nc.scalar.dma_start(out=brT, in_=brT_d)
        be = singles.tile([E, D], cdt)
        nc.scalar.dma_start(out=be, in_=be_d)
        # xT: each token-half split across two DMA lanes (HWDGE ring + a
        # SWDGE queue) so the router's first half lands in ~half the time
        xT = singles.tile([P, TH, DT, THT], cdt)
        hd = 6
        nc.sync.dma_start(out=xT[:, 0, :hd], in_=xT_d[:, 0, :hd])
        nc.gpsimd.dma_start(out=xT[:, 0, hd:], in_=xT_d[:, 0, hd:])
        nc.scalar.dma_start(out=xT[:, 1, :hd], in_=xT_d[:, 1, :hd])
        nc.gpsimd.dma_start(out=xT[:, 1, hd:], in_=xT_d[:, 1, hd:])
        ident = singles.tile([P, P], cdt)
        make_identity(nc, ident)
        ones18 = singles.tile([1, E], f32)
        nc.vector.memset(ones18, 1.0)
        ones81 = singles.tile([E, 1], f32)
        nc.vector.memset(ones81, 1.0)

        acc = singles.tile([P, TT, D], f32)
        probs = singles.tile([P, TT, E], f32)
        probsT = singles.tile([E, TT, P], cdt)
        zT = singles.tile([E, TT, P], f32)
        rsb8 = singles.tile([E, TT, P], f32)

        # ---- Router (expert-major orientation, N=512 matmuls) ----
        # logitsT[e, t] accumulates in PSUM with W_router as the stationary;
        # softmax runs along the partition axis: exp via ACT (b_router folded
        # into the bias operand), sum via GPSIMD partition-reduce, reciprocal
        # on DVE, then a stride-0 SBUF DMA broadcasts 1/sum to all 8 rows.
        for th in range(TH):
            t4 = slice(th * (TT // TH), (th + 1) * (TT // TH))
            pr = prout.tile([E, THT], f32, tag="pr")
            for dt_ in range(DT):
                nc.tensor.matmul(
                    pr, WrT[:, dt_, :], xT[:, th, dt_, :],
                    start=(dt_ == 0), stop=(dt_ == DT - 1),
                )
            nc.scalar.activation(
                out=zT[:, t4, :].rearrange("e a b -> e (a b)"), in_=pr,
                func=mybir.ActivationFunctionType.Exp, bias=brT, scale=1.0,
            )
            # sum over the 8 expert rows via a K=8 matmul (GPSIMD's C-axis
            # reduce takes ~65us for this shape — PE does it in ~1us)
            ps = prout.tile([1, THT], f32, tag="ps")
            nc.tensor.matmul(
                ps, ones81, zT[:, t4, :].rearrange("e a b -> e (a b)"),
                start=True, stop=True,
            )
            ssum = small.tile([1, THT], f32, tag="ssum")
            nc.vector.tensor_copy(ssum, ps)
            # broadcast sums [1, 512] -> [8, 512] via a K=1 rank-1 matmul,
            # then take the reciprocal across all 8 rows at once
            pb8 = prout.tile([E, THT], f32, tag="pr")
            nc.tensor.matmul(pb8, ones18, ssum, start=True, stop=True)
            nc.vector.reciprocal(
                rsb8[:, t4, :].rearrange("e a b -> e (a b)"), pb8
            )
            nc.vector.tensor_tensor(
                out=probsT[:, t4, :].rearrange("e a b -> e (a b)"),
                in0=zT[:, t4, :].rearrange("e a b -> e (a b)"),
                in1=rsb8[:, t4, :].rearrange("e a b -> e (a b)"),
                op=mybir.AluOpType.mult,
            )
        # token-major probs for the per-token combine scalars
        for tt in range(TT):
            pT = prout.tile([P, E], cdt, tag="pT")
            nc.tensor.transpose(pT, probsT[:, tt, :], ident[:E, :E])
            nc.vector.tensor_copy(probs[:, tt, :], pT)

        # ---- Bias fold: acc[t, f] = sum_e probs[t, e] * b_experts[e, f] ----
        for tt in range(TT):
            for fh in range(FH):
                pb = pbias.tile([P, FN], f32, tag="pb")
                nc.tensor.matmul(
                    pb, probsT[:, tt, :], be[:, ts(fh, FN)], start=True, stop=True
                )
                nc.vector.tensor_copy(acc[:, tt, ts(fh, FN)], pb)

        # ---- Experts: stream W, accumulate weighted outputs ----
        out_dst = out_d.rearrange("(tt p) f -> p tt f", p=P)
        half = DT // 2
        for e in range(E):
            w = wpool.tile([P, DT, D], cdt, tag="w")
            if e == 0:
                # first expert gates the steady-state PE stream: 3 lanes
                nc.sync.dma_start(out=w[:, :3], in_=Wt_d[e, :, :3])
                nc.scalar.dma_start(out=w[:, 3:6], in_=Wt_d[e, :, 3:6])
                nc.gpsimd.dma_start(out=w[:, 6:], in_=Wt_d[e, :, 6:])
            else:
                # steady state: split across both HWDGE rings
                nc.sync.dma_start(out=w[:, :half, :], in_=Wt_d[e, :, :half, :])
                nc.scalar.dma_start(out=w[:, half:, :], in_=Wt_d[e, :, half:, :])
            for tt in range(TT):
                # one stationary load serves both output halves: accumulate
                # the fh=0 and fh=1 PSUM groups side by side per d-tile
                pe0 = ppool.tile([P, FN], f32, tag="pe0")
                pe1 = ppool.tile([P, FN], f32, tag="pe1")
                for dt_ in range(DT):
                    lhsT = xT[:, tt // (TT // TH), dt_, ts(tt % (TT // TH), P)]
                    st = dt_ == 0
                    sp = dt_ == DT - 1
                    nc.tensor.matmul(
                        pe0, lhsT, w[:, dt_, 0:FN], start=st, stop=sp
                    )
                    nc.tensor.matmul(
                        pe1, lhsT, w[:, dt_, FN : 2 * FN], start=st, stop=sp
                    )
                for fh, pe_ in ((0, pe0), (1, pe1)):
                    # acc = psum * probs[:, e] + acc  (one fused DVE op)
                    nc.vector.scalar_tensor_tensor(
                        out=acc[:, tt, ts(fh, FN)],
                        in0=pe_,
                        scalar=probs[:, tt, e : e + 1],
                        in1=acc[:, tt, ts(fh, FN)],
                        op0=mybir.AluOpType.mult,
                        op1=mybir.AluOpType.add,
                    )
                    if e == E - 1:
                        # final expert: stream each finished half-tile out now
                        # so stores overlap the remaining compute
                        hwdge[fh].dma_start(
                            out=out_dst[:, tt, ts(fh, FN)],
                            in_=acc[:, tt, ts(fh, FN)],
                        )

    nc.compile()
    return nc


def prep_inputs(x, W_experts, b_experts, W_router, b_router, mode=MODE):
    """Host-side marshalling: shard tokens, transpose so the contraction dim
    is DMA-contiguous onto SBUF partitions, cast to the compute dtype."""
    ndt = _np_dt(mode)
    x = np.asarray(x, dtype=np.float32).reshape(B * S, D)
    # [E, D_out, D_in] -> transposed + tiled to [E, P, DT, D_out] so each SBUF
    # partition reads one contiguous 16KB chunk per DMA
    Wt = np.ascontiguousarray(
        np.asarray(W_experts, dtype=np.float32)
        .transpose(0, 2, 1)            # [E, D_in, D_out]
        .reshape(E, DT, P, D)
        .transpose(0, 2, 1, 3)         # [E, P, DT, D_out]
    ).astype(ndt)
    WrT = np.ascontiguousarray(
        np.asarray(W_router, dtype=np.float32)
        .T.reshape(DT, P, E)
        .transpose(1, 0, 2)            # [P, DT, E]
    ).astype(ndt)
    be = np.asarray(b_experts, dtype=np.float32).astype(ndt)
    brT = np.asarray(b_router, dtype=np.float32).reshape(E, 1)
    TH, THT = 2, T // 2
    in_maps = []
    for c in range(N_CORES):
        xT = np.ascontiguousarray(
            x[c * T : (c + 1) * T, :]
            .T.reshape(DT, P, TH, THT)
            .transpose(1, 2, 0, 3)     # [P, TH, DT, THT]
        ).astype(ndt)
        in_maps.append({"xT": xT, "Wt": Wt, "be": be, "WrT": WrT, "brT": brT})
    return in_maps


_BUILT = {}


def get_built(mode=MODE):
    if mode not in _BUILT:
        _BUILT[mode] = build(mode)
    return _BUILT[mode]


def wait_device_ready(max_tries=8, sleep_s=20):
    """Poke the axon-tunneled devices until they respond. A crashed prior
    process can leave the remote exec unit wedged for a minute or two;
    the terminal recycles it on subsequent connection attempts."""
    import time

    import jax
    import jax.numpy as jnp

    for attempt in range(max_tries):
        try:
            devs = jax.devices()
            for d in devs[:1]:
                a = jax.device_put(jnp.ones((2, 2)), d)
                np.asarray(a)
            return True
        except Exception as exc:  # noqa: BLE001
            if attempt == max_tries - 1:
                raise
            print(f"device not ready (attempt {attempt + 1}): {exc}; retrying")
            time.sleep(sleep_s)
    return False


def run_spmd(in_maps, mode=MODE, **kwargs):
    nc = get_built(mode)
    wait_device_ready()
    try:
        return run_bass_kernel_spmd(
            nc, in_maps, core_ids=list(range(N_CORES)), **kwargs
        )
    except Exception as exc:  # noqa: BLE001
        print(f"run_bass_kernel_spmd failed ({exc}); retrying once after re-poke")
        wait_device_ready()
        return run_bass_kernel_spmd(
            nc, in_maps, core_ids=list(range(N_CORES)), **kwargs
        )


def kernel(x, W_experts, b_experts, W_router, b_router):
    in_maps = prep_inputs(x, W_experts, b_experts, W_router, b_router)
    res = run_spmd(in_maps)
    out = np.concatenate(
        [np.asarray(res.results[c]["out"], dtype=np.float32) for c in range(N_CORES)],
        axis=0,
    )
    return out.reshape(B, S, D)


# revision 27
# speedup vs baseline: 1.0360x; 1.0360x over previous
"""MoE ExpertLayer kernel for Trainium2 (8 NeuronCores, data-parallel over tokens).

Reference computation (B=4, S=2048, D=1024, E=8):
    logits  = x @ W_router.T + b_router          # [B,S,E]
    probs   = softmax(logits, axis=-1)
    y_e     = x @ W_experts[e].T + b_experts[e]  # all experts, dense
    out     = sum_e probs[..., e] * y_e          # [B,S,D]

Sharding: data-parallel over the flattened token axis (8192 tokens -> 1024
tokens per core). Every core receives the full (transposed) expert weights and
computes its token shard end-to-end; no collectives are needed.

Per-core dataflow:
  - xT [D, T] resident in SBUF; expert weights streamed one expert at a time
    as WtT[e] = W_experts[e].T (so the contraction dim d lands on SBUF
    partitions for both matmul operands with contiguous DMA).
  - Router: 8 accumulating matmuls per token tile -> PSUM [128 tok, 8 e],
    + K=1 ones-matmul to add b_router; softmax via DVE reduce_max(negate) +
    ACT Exp(bias=-max, accum_out=sum) + DVE reciprocal + tensor_scalar_mul.
  - Bias fold: out bias term sum_e probs[t,e]*b_e[f] is a K=8 matmul
    (probs.T as stationary) accumulated straight into the output accumulator.
  - Experts: psum[t=128, f=512] accumulates 8 d-tile matmuls; the combine
    acc = psum * probs[:,e] + acc is one fused DVE scalar_tensor_tensor op.
"""

import os
import sys

for _p in ("/opt/trn_rl_repo", "/root/.axon_site/_ro/trn_rl_repo"):
    if os.path.isdir(_p) and _p not in sys.path:
        sys.path.insert(0, _p)

from contextlib import ExitStack

import ml_dtypes
import numpy as np

import concourse.bass as bass
import concourse.mybir as mybir
import concourse.tile as tile
from concourse import bacc
from concourse.bass import ts
from concourse.bass_utils import run_bass_kernel_spmd
from concourse.masks import make_identity

B, S, D, E = 4, 2048, 1024, 8
N_CORES = 8
T = B * S // N_CORES  # tokens per core = 1024
P = 128               # partitions
TT = T // P           # token tiles per core = 8
DT = D // P           # contraction tiles = 8
FN = 512              # matmul moving free dim (one PSUM bank of fp32)
FH = D // FN          # output column halves = 2

MODE = os.environ.get("KERNEL_MODE", "bf16")  # bf16 | f32r | f32


def _compute_dt(mode):
    return {
        "bf16": mybir.dt.bfloat16,
        "f32r": mybir.dt.float32r,
        "f32": mybir.dt.float32,
    }[mode]


def _np_dt(mode):
    return {"bf16": ml_dtypes.bfloat16, "f32r": np.float32, "f32": np.float32}[mode]


def build(mode=MODE):
    """Build the per-core Bass/Tile program (identical SPMD program on all cores)."""
    cdt = _compute_dt(mode)
    f32 = mybir.dt.float32

    nc = bacc.Bacc("TRN2", target_bir_lowering=False, debug=False)

    # Inputs are pre-tiled on the host to [partition, ..., d-tile, ...] so
    # every DMA reads long contiguous per-partition chunks — the naive
    # [D, ...] layout yields 2KB strided descriptors that throttle a HWDGE
    # queue. xT is additionally split by token half so the router can start
    # as soon as the first half lands.
    TH = 2          # token halves per core
    THT = T // TH   # 512 tokens per half
    xT_d = nc.dram_tensor("xT", [P, TH, DT, THT], cdt, kind="ExternalInput").ap()
    Wt_d = nc.dram_tensor("Wt", [E, P, DT, D], cdt, kind="ExternalInput").ap()
    be_d = nc.dram_tensor("be", [E, D], cdt, kind="ExternalInput").ap()
    WrT_d = nc.dram_tensor("WrT", [P, DT, E], cdt, kind="ExternalInput").ap()
    brT_d = nc.dram_tensor("brT", [E, 1], f32, kind="ExternalInput").ap()
    out_d = nc.dram_tensor("out", [T, D], f32, kind="ExternalOutput").ap()

    with tile.TileContext(nc) as tc, ExitStack() as ctx:
        singles = ctx.enter_context(tc.tile_pool(name="singles", bufs=1))
        wpool = ctx.enter_context(tc.tile_pool(name="wpool", bufs=3))
        small = ctx.enter_context(tc.tile_pool(name="small", bufs=4))
        ppool = ctx.enter_context(tc.tile_pool(name="psum_e", bufs=2, space="PSUM"))
        pbias = ctx.enter_context(tc.tile_pool(name="psum_b", bufs=1, space="PSUM"))
        prout = ctx.enter_context(tc.tile_pool(name="psum_r", bufs=1, space="PSUM"))

        # Two HWDGE rings (sync=SP, scalar=ACT): spread big transfers across
        # both — a single ring saturates around ~120 GB/s for these patterns.
        hwdge = [nc.sync, nc.scalar]

        # Resident tensors. Tiny router tensors go first (they gate the router
        # phase), then the xT token-halves — one per ring — then weights.
        WrT = singles.tile([P, DT, E], cdt)
        nc.sync.dma_start(out=WrT, in_=WrT_d)
        brT = singles.tile([E, 1], f32)
        nc.scalar.dma_start(out=brT, in_=brT_d)
        be = singles.tile([E, D], cdt)
        nc.scalar.dma_start(out=be, in_=be_d)
        # xT token-halves, one per HWDGE ring
        xT = singles.tile([P, TH, DT, THT], cdt)
        nc.sync.dma_start(out=xT[:, 0], in_=xT_d[:, 0])
        nc.scalar.dma_start(out=xT[:, 1], in_=xT_d[:, 1])
        identf = singles.tile([P, P], f32)
        make_identity(nc, identf)

        acc = singles.tile([P, TT, D], f32)
        probs = singles.tile([P, TT, E], f32)
        probsT = singles.tile([E, TT, P], cdt)
        logitsT = singles.tile([E, TT, P], f32)

        # ---- Router ----
        # logitsT[e, t] accumulates in PSUM with W_router as the stationary
        # (16 N=512 matmuls instead of 64 N=8 ones); b_router is added on the
        # ACT copy out of PSUM (per-partition bias). Each token tile is then
        # transposed back to [tok, e] on the PE and soft-maxed with cheap
        # [128, 1] per-token reductions; the bias-fold matmuls interleave
        # per-tt so the PE has useful work while the other xT half lands.
        for th in range(TH):
            t4 = slice(th * (TT // TH), (th + 1) * (TT // TH))
            pr = prout.tile([E, THT], f32, tag="pr")
            for dt_ in range(DT):
                nc.tensor.matmul(
                    pr, WrT[:, dt_, :], xT[:, th, dt_, :],
                    start=(dt_ == 0), stop=(dt_ == DT - 1),
                )
            nc.scalar.activation(
                out=logitsT[:, t4, :].rearrange("e a b -> e (a b)"), in_=pr,
                func=mybir.ActivationFunctionType.Identity, bias=brT, scale=1.0,
            )
            for tt in range(th * (TT // TH), (th + 1) * (TT // TH)):
                pT = prout.tile([P, E], f32, tag="pT")
                nc.tensor.transpose(pT, logitsT[:, tt, :], identf[:E, :E])
                negmax = small.tile([P, 1], f32, tag="negmax")
                nc.vector.reduce_max(
                    out=negmax, in_=pT, axis=mybir.AxisListType.X, negate=True
                )
                z = small.tile([P, E], f32, tag="z")
                ssum = small.tile([P, 1], f32, tag="ssum")
                nc.scalar.activation(
                    out=z, in_=pT, func=mybir.ActivationFunctionType.Exp,
                    bias=negmax, scale=1.0, accum_out=ssum,
                )
                rec = small.tile([P, 1], f32, tag="rec")
                nc.vector.reciprocal(rec, ssum)
                nc.vector.tensor_scalar_mul(probs[:, tt, :], z, rec)
                # probs.T (bf16) for the bias-fold matmul
                pT2 = prout.tile([E, P], f32, tag="pT2")
                nc.tensor.transpose(pT2, probs[:, tt, :], identf)
                nc.vector.tensor_copy(probsT[:, tt, :], pT2)
                # bias fold: acc[t, f] = sum_e probs[t, e] * b_experts[e, f]
                for fh in range(FH):
                    pb = pbias.tile([P, FN], f32, tag="pb")
                    nc.tensor.matmul(
                        pb, probsT[:, tt, :], be[:, ts(fh, FN)],
                        start=True, stop=True,
                    )
                    nc.vector.tensor_copy(acc[:, tt, ts(fh, FN)], pb)

        # ---- Experts: stream W, accumulate weighted outputs ----
        out_dst = out_d.rearrange("(tt p) f -> p tt f", p=P)
        half = DT // 2
        for e in range(E):
            w = wpool.tile([P, DT, D], cdt, tag="w")
            # split each expert's 2MB across both HWDGE rings
            nc.sync.dma_start(out=w[:, :half, :], in_=Wt_d[e, :, :half, :])
            nc.scalar.dma_start(out=w[:, half:, :], in_=Wt_d[e, :, half:, :])
            for tt in range(TT):
                # one stationary load serves both output halves: accumulate
                # the fh=0 and fh=1 PSUM groups side by side per d-tile
                pe0 = ppool.tile([P, FN], f32, tag="pe0")
                pe1 = ppool.tile([P, FN], f32, tag="pe1")
                for dt_ in range(DT):
                    lhsT = xT[:, tt // (TT // TH), dt_, ts(tt % (TT // TH), P)]
                    st = dt_ == 0
                    sp = dt_ == DT - 1
                    nc.tensor.matmul(
                        pe0, lhsT, w[:, dt_, 0:FN], start=st, stop=sp
                    )
                    nc.tensor.matmul(
                        pe1, lhsT, w[:, dt_, FN : 2 * FN], start=st, stop=sp
                    )
                for fh, pe_ in ((0, pe0), (1, pe1)):
                    # acc = psum * probs[:, e] + acc  (one fused DVE op)
                    nc.vector.scalar_tensor_tensor(
                        out=acc[:, tt, ts(fh, FN)],
                        in0=pe_,
                        scalar=probs[:, tt, e : e + 1],
                        in1=acc[:, tt, ts(fh, FN)],
                        op0=mybir.AluOpType.mult,
                        op1=mybir.AluOpType.add,
                    )
                    if e == E - 1:
                        # final expert: stream each finished half-tile out now
                        # so stores overlap the remaining compute
                        hwdge[fh].dma_start(
                            out=out_dst[:, tt, ts(fh, FN)],
                            in_=acc[:, tt, ts(fh, FN)],
                        )

    nc.compile()
    return nc


def prep_inputs(x, W_experts, b_experts, W_router, b_router, mode=MODE):
    """Host-side marshalling: shard tokens, transpose so the contraction dim
    is DMA-contiguous onto SBUF partitions, cast to the compute dtype."""
    ndt = _np_dt(mode)
    x = np.asarray(x, dtype=np.float32).reshape(B * S, D)
    # [E, D_out, D_in] -> transposed + tiled to [E, P, DT, D_out] so each SBUF
    # partition reads one contiguous 16KB chunk per DMA
    Wt = np.ascontiguousarray(
        np.asarray(W_experts, dtype=np.float32)
        .transpose(0, 2, 1)            # [E, D_in, D_out]
        .reshape(E, DT, P, D)
        .transpose(0, 2, 1, 3)         # [E, P, DT, D_out]
    ).astype(ndt)
    WrT = np.ascontiguousarray(
        np.asarray(W_router, dtype=np.float32)
        .T.reshape(DT, P, E)
        .transpose(1, 0, 2)            # [P, DT, E]
    ).astype(ndt)
    be = np.asarray(b_experts, dtype=np.float32).astype(ndt)
    brT = np.asarray(b_router, dtype=np.float32).reshape(E, 1)
    TH, THT = 2, T // 2
    in_maps = []
    for c in range(N_CORES):
        xT = np.ascontiguousarray(
            x[c * T : (c + 1) * T, :]
            .T.reshape(DT, P, TH, THT)
            .transpose(1, 2, 0, 3)     # [P, TH, DT, THT]
        ).astype(ndt)
        in_maps.append({"xT": xT, "Wt": Wt, "be": be, "WrT": WrT, "brT": brT})
    return in_maps


_BUILT = {}


def get_built(mode=MODE):
    if mode not in _BUILT:
        _BUILT[mode] = build(mode)
    return _BUILT[mode]


def wait_device_ready(max_tries=8, sleep_s=20):
    """Poke the axon-tunneled devices until they respond. A crashed prior
    process can leave the remote exec unit wedged for a minute or two;
    the terminal recycles it on subsequent connection attempts."""
    import time

    import jax
    import jax.numpy as jnp

    for attempt in range(max_tries):
        try:
            devs = jax.devices()
            for d in devs[:1]:
                a = jax.device_put(jnp.ones((2, 2)), d)
                np.asarray(a)
            return True
        except Exception as exc:  # noqa: BLE001
            if attempt == max_tries - 1:
                raise
            print(f"device not ready (attempt {attempt + 1}): {exc}; retrying")
            time.sleep(sleep_s)
    return False


def run_spmd(in_maps, mode=MODE, **kwargs):
    nc = get_built(mode)
    wait_device_ready()
    try:
        return run_bass_kernel_spmd(
            nc, in_maps, core_ids=list(range(N_CORES)), **kwargs
        )
    except Exception as exc:  # noqa: BLE001
        print(f"run_bass_kernel_spmd failed ({exc}); retrying once after re-poke")
        wait_device_ready()
        return run_bass_kernel_spmd(
            nc, in_maps, core_ids=list(range(N_CORES)), **kwargs
        )


def kernel(x, W_experts, b_experts, W_router, b_router):
    in_maps = prep_inputs(x, W_experts, b_experts, W_router, b_router)
    res = run_spmd(in_maps)
    out = np.concatenate(
        [np.asarray(res.results[c]["out"], dtype=np.float32) for c in range(N_CORES)],
        axis=0,
    )
    return out.reshape(B, S, D)
